# revision 1
# baseline (speedup 1.0000x reference)
"""ChildSum TreeLSTM (N=8192 nodes, 4-ary static heap tree, H=256, D=300) on 8 trn2 NeuronCores.

Strategy
--------
The tree is static: node i's children are 4i+1..4i+4 (clipped at N). The reverse
scan (children before parents) is equivalent to processing the tree level by
level, bottom-up; nodes within a level are independent, so each level is a
batched LSTM cell (matmuls + elementwise).

Sharding: the 256 level-4 subtrees are partitioned across the 8 cores (balanced
by the number of *internal* level-6 descendants, which determines level-7 leaf
count). Each core processes its forest fully locally — children of a sorted node
range are contiguous in the next level's sorted array, so the recurrence needs
no gathers and no cross-core communication. Cores output their 32 level-4 root
(h, c) states; the tiny top of the tree (levels 3..0, 85 nodes) plus the final
log_softmax run on the host in numpy.

On-device layout: everything is transposed — feature dim on SBUF partitions
(256 features = 2 halves of 128), nodes along the free axis. The child-h sums
and per-child forget gates then become strided slicing along the free axis, and
the x-side / h-side gate projections accumulate into the same PSUM tile.
Biases (bx + bh, zeros in practice) are folded into an extra ones-row of the
x-side matmul, so pad columns (zero x) self-compute to h = c = 0.
"""

import numpy as np
import ml_dtypes

BF16 = ml_dtypes.bfloat16

N = 8192
H = 256
D = 300
K = 4
OUT = 4
NCORES = 8
L7P = 384           # padded level-7 columns per core (4 * IPMAX)
IPMAX = 96          # max internal level-6 nodes per core
KDIM = 304          # padded contraction rows of xt/wx (300 emb + 1 ones + pad)
KUSE = 301          # rows actually used in matmuls
XCOLS = L7P + 512 + 128 + 32   # 1056 per-core node columns: [L7 | L6 | L5 | L4]

GATE_MAP = [0, 2, 3, 1]  # our gate order [i, o, u, f] -> reference gate indices

F32 = np.float32


def _build_plan():
    """Assign the 256 level-4 subtrees to 8 cores; build per-core column maps."""
    # w(u) = number of internal (has-children) level-6 descendants of L4 node u.
    # Full-weight subtrees (w=16) are u in [85, 127); u=127 has w=11; rest 0.
    full = list(range(85, 127))                               # 42 subtrees
    lights = list(range(128, 341))                            # 213 subtrees
    heavy_counts = [6, 6, 5, 5, 5, 5, 5, 5]                   # sums to 42
    light_counts = [26, 26, 26, 27, 27, 27, 27, 27]           # sums to 213
    cores = []
    hpos = 0
    lpos = 0
    for c in range(NCORES):
        hs = full[hpos:hpos + heavy_counts[c]]
        hpos += heavy_counts[c]
        if c == 2:
            hs = hs + [127]                                   # w sums: 96,96,91,80*5
        ls = lights[lpos:lpos + light_counts[c]]
        lpos += light_counts[c]
        cores.append(sorted(hs + ls))
    all_l4 = sorted(u for cs in cores for u in cs)
    assert all_l4 == list(range(85, 341)), "L4 assignment must partition [85, 341)"

    plan = []
    for c in range(NCORES):
        l4 = cores[c]
        assert len(l4) == 32
        l5 = [4 * u + 1 + k for u in l4 for k in range(K)]
        l6 = [4 * v + 1 + k for v in l5 for k in range(K)]
        wc = sum(1 for x in l6 if x < 2048)
        assert wc <= IPMAX
        l7 = []
        for x in l6[:wc]:
            for k in range(K):
                ch = 4 * x + 1 + k
                l7.append(ch if ch < N else -1)
        l7 += [-1] * (L7P - len(l7))
        cols = np.array(l7 + l6 + l5 + l4, dtype=np.int64)
        assert cols.shape == (XCOLS,)
        plan.append((cols, wc, np.array(l4, dtype=np.int64)))
    return plan


_PLAN = _build_plan()

# chunk schedule: (xoff, ncols, ip, child_level, child_col_off, out_level, out_off)
# child/out levels refer to state buffers keyed 7, 6, 5, 4. L5 is split in two
# so its halves chain off the two independent L6 chunks — the scheduler can
# overlap one chain's elementwise tail with the other chain's matmuls.
_CHUNKS = [
    (0,    256, 0,   None, 0,   7, 0),     # L7 leaves, part A
    (256,  128, 0,   None, 0,   7, 256),   # L7 leaves, part B
    (640,  256, 0,   None, 0,   6, 256),   # L6 leaf-only half
    (384,  256, 96,  7,    0,   6, 0),     # L6 internal half
    (896,  128, 128, 6,    0,   5, 0),     # L5
    (1024, 32,  32,  5,    0,   4, 0),     # L4
]
_STATE_COLS = {7: L7P, 6: 512, 5: 128, 4: 32}


def _static_tree():
    idx = np.arange(N)[:, None] * K + 1 + np.arange(K)[None, :]
    mask = (idx < N).astype(F32)
    idx = np.where(idx < N, idx, 0).astype(np.int32)
    return idx, mask


_STATIC_IDX, _STATIC_MASK = _static_tree()


def _pack_weights(Wx, bx, Wh, bh):
    wx = np.zeros((KDIM, 4 * H), dtype=F32)  # cast to bf16 at return
    for g, rg in enumerate(GATE_MAP):
        wx[:D, H * g:H * (g + 1)] = np.asarray(Wx[rg], dtype=F32).T
        wx[D, H * g:H * (g + 1)] = np.asarray(bx[rg], dtype=F32) + np.asarray(bh[rg], dtype=F32)
    wh = np.zeros((H, 3 * H), dtype=F32)
    for g, rg in enumerate([0, 2, 3]):  # i, o, u
        wh[:, H * g:H * (g + 1)] = np.asarray(Wh[rg], dtype=F32).T
    whf = np.ascontiguousarray(np.asarray(Wh[1], dtype=F32).T)
    return wx.astype(BF16), wh, whf


def _pack_xt(xs, emb_table):
    X = np.asarray(emb_table, dtype=F32)[np.asarray(xs)]
    xts = []
    for cols, _, _ in _PLAN:
        xt = np.zeros((KDIM, XCOLS), dtype=F32)
        real = cols >= 0
        xt[:D, real] = X[cols[real]].T
        xt[D, real] = 1.0
        xts.append(xt.astype(BF16))
    return xts


def _sigmoid(x):
    return (1.0 / (1.0 + np.exp(-x))).astype(F32)


def _host_top(Hbuf, Cbuf, xs, emb_table, Wx, bx, Wh, bh):
    """Compute tree levels 3..0 (nodes 0..84) on the host, numpy fp32."""
    Wx = np.asarray(Wx, dtype=F32)
    bx = np.asarray(bx, dtype=F32)
    Wh = np.asarray(Wh, dtype=F32)
    bh = np.asarray(bh, dtype=F32)
    emb = np.asarray(emb_table, dtype=F32)
    xs = np.asarray(xs)
    for lo, hi in [(21, 85), (5, 21), (1, 5), (0, 1)]:
        ids = np.arange(lo, hi)
        Xl = emb[xs[ids]]                                   # [n, D]
        gx = np.einsum('ghd,nd->ngh', Wx, Xl).astype(F32) + bx
        cidx = ids[:, None] * K + 1 + np.arange(K)[None, :]  # all valid (< 341)
        Hc = Hbuf[cidx]
        Cc = Cbuf[cidx]
        hs = Hc.sum(1)
        ig = _sigmoid(gx[:, 0] + hs @ Wh[0].T + bh[0])
        og = _sigmoid(gx[:, 2] + hs @ Wh[2].T + bh[2])
        ug = np.tanh(gx[:, 3] + hs @ Wh[3].T + bh[3]).astype(F32)
        f = _sigmoid(gx[:, 1][:, None, :] + Hc @ Wh[1].T + bh[1])
        cc = ig * ug + (f * Cc).sum(1)
        hh = og * np.tanh(cc).astype(F32)
        Hbuf[ids] = hh
        Cbuf[ids] = cc
    return Hbuf[0]


def _log_softmax(x):
    m = np.max(x)
    e = np.exp(x - m)
    return (x - m - np.log(e.sum())).astype(F32)


def simulate_cores_numpy(inputs):
    """Numpy emulation of the exact device data layout & chunk schedule.

    Returns (Hbuf, Cbuf) filled for nodes [85, 341) — for validating the plan
    against the reference without hardware.
    """
    xs = np.asarray(inputs["xs"])
    wx, wh, whf = _pack_weights(inputs["Wx"], inputs["bx"], inputs["Wh"], inputs["bh"])
    xts = _pack_xt(xs, inputs["emb_table"])
    Hbuf = np.zeros((341, H), dtype=F32)
    Cbuf = np.zeros((341, H), dtype=F32)
    for c in range(NCORES):
        cols, wc, l4 = _PLAN[c]
        xt = xts[c]
        state_h = {lv: np.zeros((H, n), dtype=F32) for lv, n in _STATE_COLS.items()}
        state_c = {lv: np.zeros((H, n), dtype=F32) for lv, n in _STATE_COLS.items()}
        for (xoff, nc_, ip, child, coff, outlv, ooff) in _CHUNKS:
            xk = xt[:KUSE, xoff:xoff + nc_].astype(F32)         # [301, nc]
            G = wx[:KUSE].astype(F32).T @ xk                    # [1024, nc]
            gi = G[0:H]
            go = G[H:2 * H]
            gu = G[2 * H:3 * H]
            gf = G[3 * H:4 * H]
            if ip > 0:
                ch_h = state_h[child][:, coff:coff + 4 * ip]    # [H, 4ip]
                ch_c = state_c[child][:, coff:coff + 4 * ip]
                hs = ch_h.reshape(H, ip, K).sum(axis=2)         # [H, ip]
                A = wh.T @ hs                                   # [768, ip]
                gi[:, :ip] += A[0:H]
                go[:, :ip] += A[H:2 * H]
                gu[:, :ip] += A[2 * H:3 * H]
                Fp = whf.T @ ch_h                               # [H, 4ip]
                FA = Fp + np.repeat(gf[:, :ip], K, axis=1)
                FS = _sigmoid(FA) * ch_c
                csum = FS.reshape(H, ip, K).sum(axis=2)
            ig = _sigmoid(gi)
            og = _sigmoid(go)
            ug = np.tanh(gu).astype(F32)
            cc = ig * ug
            if ip > 0:
                cc[:, :ip] += csum
            hh = og * np.tanh(cc).astype(F32)
            state_h[outlv][:, ooff:ooff + nc_] = hh
            state_c[outlv][:, ooff:ooff + nc_] = cc
        Hbuf[l4] = state_h[4].T
        Cbuf[l4] = state_c[4].T
    return Hbuf, Cbuf


# ----------------------------------------------------------------------------
# Bass device program
# ----------------------------------------------------------------------------

_COMPILED = None


def _build_device_program():
    import contextlib

    import concourse.bacc as bacc
    import concourse.tile as tile
    import concourse.mybir as mybir

    f32 = mybir.dt.float32
    f32r = mybir.dt.float32r
    bf16 = mybir.dt.bfloat16
    Sig = mybir.ActivationFunctionType.Sigmoid
    Tanh = mybir.ActivationFunctionType.Tanh

    nc = bacc.Bacc("TRN2", target_bir_lowering=False, debug=False,
                   num_devices=NCORES)

    def mm(out, lhsT, rhs, **kw):
        # float32r operands: same fp32 bytes, single-pass reduced-precision
        # multiply (vs fp32's two half-speed passes + double weight load).
        nc.tensor.matmul(out, lhsT, rhs, **kw)

    xt_d = nc.dram_tensor("xt", [KDIM, XCOLS], bf16, kind="ExternalInput")
    wx_d = nc.dram_tensor("wx", [KDIM, 4 * H], bf16, kind="ExternalInput")
    wh_d = nc.dram_tensor("wh", [H, 3 * H], f32r, kind="ExternalInput")
    whf_d = nc.dram_tensor("whf", [H, H], f32r, kind="ExternalInput")
    out_h_d = nc.dram_tensor("out_h", [128, 2, 32], f32r, kind="ExternalOutput")
    out_c_d = nc.dram_tensor("out_c", [128, 2, 32], f32, kind="ExternalOutput")

    krows = [(0, 128), (128, 256), (256, KUSE)]
    RANGES = [(0, 512), (512, 1024), (1024, XCOLS)]

    with tile.TileContext(nc) as tc:
        with contextlib.ExitStack() as ctx:
            inp = ctx.enter_context(tc.tile_pool(name="inp", bufs=1))
            st = ctx.enter_context(tc.tile_pool(name="state", bufs=1))
            wk = ctx.enter_context(tc.tile_pool(name="work", bufs=2))
            fwk = ctx.enter_context(tc.tile_pool(name="fwork", bufs=3))
            ps = ctx.enter_context(
                tc.tile_pool(name="psum", bufs=2, space="PSUM"))

            # --- inputs to SBUF, spread across otherwise-idle engines; xt is
            # split into chunk-aligned column ranges so the first level can
            # start as soon as its columns land.
            xt_s = []
            wx_s = []
            for k, (r0, r1) in enumerate(krows[:2] + [(256, 304)]):
                t = inp.tile([r1 - r0, 4 * H], bf16, tag=f"wx{k}", name=f"wx{k}")
                wx_s.append(t)
            # wx on gpsimd in column quarters, k-interleaved, so the first
            # gate's weights land quickly
            for q in range(4):
                for k, (r0, r1) in enumerate(krows[:2] + [(256, 304)]):
                    nc.gpsimd.dma_start(
                        out=wx_s[k][:, 256 * q:256 * (q + 1)],
                        in_=wx_d[r0:r1, 256 * q:256 * (q + 1)])
            wh_s = []
            whf_s = []
            for k, (r0, r1) in enumerate([(0, 128), (128, 256)]):
                t = inp.tile([128, 3 * H], f32r, tag=f"wh{k}", name=f"wh{k}")
                nc.scalar.dma_start(out=t[:], in_=wh_d[r0:r1, :])
                wh_s.append(t)
                t = inp.tile([128, H], f32r, tag=f"whf{k}", name=f"whf{k}")
                nc.scalar.dma_start(out=t[:], in_=whf_d[r0:r1, :])
                whf_s.append(t)
            for k, (r0, r1) in enumerate(krows[:2] + [(256, 304)]):
                t = inp.tile([r1 - r0, XCOLS], bf16, tag=f"xt{k}", name=f"xt{k}")
                xt_s.append(t)
            # xt on sync, range-major so the first level's columns land first
            for (a, b) in RANGES:
                for k, (r0, r1) in enumerate(krows[:2] + [(256, 304)]):
                    nc.sync.dma_start(out=xt_s[k][:, a:b], in_=xt_d[r0:r1, a:b])

            # --- persistent state + gx tiles
            SH = {lv: st.tile([128, 2, n], f32r, tag=f"h{lv}", name=f"sh{lv}")
                  for lv, n in _STATE_COLS.items()}
            SC = {lv: st.tile([128, 2, n], f32, tag=f"c{lv}", name=f"sc{lv}")
                  for lv, n in _STATE_COLS.items()}
            GX = [st.tile([128, 2, XCOLS], f32, tag=f"gx{g}", name=f"gx{g}")
                  for g in range(4)]

            # --- phase 1: all x-side gate projections in one weight-stationary
            # sweep (minimal LDWEIGHTS: 24 distinct weight tiles, loaded once),
            # drained PSUM -> SBUF by DMA.
            for m in range(8):
                g, phi = divmod(m, 2)
                col = H * g + 128 * phi
                ptiles = [
                    ps.tile([128, b - a], f32, tag="gx", name=f"pgx{m}_{i}",
                            bufs=4)
                    for i, (a, b) in enumerate(RANGES)
                ]
                for k in range(3):
                    r0, r1 = krows[k]
                    for i, (a, b) in enumerate(RANGES):
                        mm(ptiles[i][:],
                           wx_s[k][0:r1 - r0, col:col + 128],
                           xt_s[k][0:r1 - r0, a:b],
                           start=(k == 0), stop=(k == 2))
                for i, (a, b) in enumerate(RANGES):
                    if (m + i) % 2 == 0:
                        nc.scalar.copy(GX[g][:, phi, a:b], ptiles[i][:])
                    else:
                        nc.vector.tensor_copy(GX[g][:, phi, a:b], ptiles[i][:])

            # --- phase 2: levels bottom-up in chunks
            for (xoff, cn, ip, child, coff, outlv, ooff) in _CHUNKS:
                hs = None
                if ip > 0:
                    # hs = sum of the 4 child h columns per node (on the
                    # otherwise-idle gpsimd engine)
                    hs = wk.tile([128, 2, ip], f32r, tag="hs", name="hs")
                    for phi in range(2):
                        cv = SH[child][:, phi, coff:coff + 4 * ip].rearrange(
                            "p (n k) -> p n k", k=K)
                        dst = hs[:, phi, :]
                        nc.gpsimd.tensor_add(dst, cv[:, :, 0], cv[:, :, 1])
                        nc.gpsimd.tensor_add(dst, dst, cv[:, :, 2])
                        nc.gpsimd.tensor_add(dst, dst, cv[:, :, 3])

                gates = []
                for gi_, func in ((0, Sig), (1, Sig), (2, Tanh)):
                    G = wk.tile([128, 2, cn], f32, tag=f"g{gi_}", name=f"g{gi_}")
                    if ip > 0:
                        P = ps.tile([128, 2, ip], f32, tag="pa", name=f"pa{gi_}",
                                    bufs=2)
                        for phi in range(2):
                            for k in range(2):
                                mm(P[:, phi, :],
                                   wh_s[k][:, H * gi_ + 128 * phi:
                                           H * gi_ + 128 * phi + 128],
                                   hs[:, k, 0:ip],
                                   start=(k == 0), stop=(k == 1))
                        nc.vector.tensor_add(
                            P[:], P[:], GX[gi_][:, :, xoff:xoff + ip])
                        nc.scalar.activation(G[:, :, 0:ip], P[:], func)
                        if cn > ip:
                            nc.scalar.activation(
                                G[:, :, ip:cn],
                                GX[gi_][:, :, xoff + ip:xoff + cn], func)
                    else:
                        nc.scalar.activation(
                            G[:], GX[gi_][:, :, xoff:xoff + cn], func)
                    gates.append(G)
                IG, OG, UG = gates

                csum = None
                if ip > 0:
                    # f = sigmoid(gf + Whf @ h_child), per child; then
                    # csum = sum_k f_k * c_child_k
                    csum = wk.tile([128, 2, ip], f32, tag="csum", name="csum")
                    for phi in range(2):
                        Pfc = ps.tile([128, 4 * ip], f32, tag="pf",
                                      name=f"pfc{phi}")
                        for k in range(2):
                            mm(Pfc[:],
                               whf_s[k][:, 128 * phi:128 * phi + 128],
                               SH[child][:, k, coff:coff + 4 * ip],
                               start=(k == 0), stop=(k == 1))
                        pv = Pfc.rearrange("p (n k) -> p n k", k=K)
                        gfb = GX[3][:, phi, xoff:xoff + ip][:, :, None]
                        nc.vector.tensor_add(
                            pv, pv, gfb.broadcast_to([128, ip, K]))
                        FS = fwk.tile([128, 4 * ip], f32, tag="fs", name="fs")
                        nc.scalar.activation(FS[:], Pfc[:], Sig)
                        nc.vector.tensor_mul(
                            FS[:], FS[:], SC[child][:, phi, coff:coff + 4 * ip])
                        sv = FS.rearrange("p (n k) -> p n k", k=K)
                        dst = csum[:, phi, :]
                        nc.gpsimd.tensor_add(dst, sv[:, :, 0], sv[:, :, 1])
                        nc.gpsimd.tensor_add(dst, dst, sv[:, :, 2])
                        nc.gpsimd.tensor_add(dst, dst, sv[:, :, 3])

                # c = ig*ug (+ csum on internal cols); h = og*tanh(c)
                Cdst = SC[outlv][:, :, ooff:ooff + cn]
                nc.vector.tensor_mul(Cdst, IG[:], UG[:])
                if ip > 0:
                    nc.vector.tensor_add(
                        SC[outlv][:, :, ooff:ooff + ip],
                        SC[outlv][:, :, ooff:ooff + ip],
                        csum[:],
                    )
                TC = wk.tile([128, 2, cn], f32, tag="tc", name="tc")
                nc.scalar.activation(TC[:], Cdst, Tanh)
                nc.vector.tensor_mul(
                    SH[outlv][:, :, ooff:ooff + cn], OG[:], TC[:])

            nc.sync.dma_start(out=out_h_d[:], in_=SH[4][:])
            nc.sync.dma_start(out=out_c_d[:], in_=SC[4][:])

    nc.compile()
    return nc


def _get_compiled():
    global _COMPILED
    if _COMPILED is None:
        _COMPILED = _build_device_program()
    return _COMPILED


def _numpy_fallback(xs, child_idx, child_mask, emb_table, Wx, bx, Wh, bh,
                    Wout, bout):
    """Exact sequential scan in numpy; only used if the tree is not the
    expected static 4-ary heap."""
    X = np.asarray(emb_table, dtype=F32)[np.asarray(xs)]
    Wx = np.asarray(Wx, dtype=F32)
    Wh = np.asarray(Wh, dtype=F32)
    bx = np.asarray(bx, dtype=F32)
    bh = np.asarray(bh, dtype=F32)
    gx = np.einsum('ghd,nd->ngh', Wx, X).astype(F32) + bx
    Hb = np.zeros((N, H), dtype=F32)
    Cb = np.zeros((N, H), dtype=F32)
    ci = np.asarray(child_idx)
    cm = np.asarray(child_mask, dtype=F32)
    for i in range(N - 1, -1, -1):
        idx = ci[i]
        m = cm[i][:, None]
        Hc = Hb[idx] * m
        Cc = Cb[idx] * m
        hs = Hc.sum(0)
        g = gx[i]
        ig = _sigmoid(g[0] + Wh[0] @ hs + bh[0])
        og = _sigmoid(g[2] + Wh[2] @ hs + bh[2])
        ug = np.tanh(g[3] + Wh[3] @ hs + bh[3]).astype(F32)
        f = _sigmoid(g[1] + Hc @ Wh[1].T + bh[1])
        c = ig * ug + (f * Cc).sum(0)
        Hb[i] = og * np.tanh(c).astype(F32)
        Cb[i] = c
    logits = np.asarray(Wout, dtype=F32) @ Hb[0] + np.asarray(bout, dtype=F32)
    return _log_softmax(logits)


def kernel(xs, child_idx, child_mask, emb_table, Wx, bx, Wh, bh, Wout, bout):
    xs = np.asarray(xs)
    if not (np.array_equal(np.asarray(child_idx), _STATIC_IDX)
            and np.array_equal(np.asarray(child_mask, dtype=F32), _STATIC_MASK)):
        return _numpy_fallback(xs, child_idx, child_mask, emb_table, Wx, bx,
                               Wh, bh, Wout, bout)

    from concourse.bass_utils import run_bass_kernel_spmd

    wx, wh, whf = _pack_weights(Wx, bx, Wh, bh)
    xts = _pack_xt(xs, emb_table)
    in_maps = [
        {"xt": xts[c], "wx": wx, "wh": wh, "whf": whf} for c in range(NCORES)
    ]
    nc = _get_compiled()
    res = run_bass_kernel_spmd(nc, in_maps, core_ids=list(range(NCORES)))

    Hbuf = np.zeros((341, H), dtype=F32)
    Cbuf = np.zeros((341, H), dtype=F32)
    for c in range(NCORES):
        _, _, l4 = _PLAN[c]
        oh = res.results[c]["out_h"]   # [128, 2, 32]
        oc = res.results[c]["out_c"]
        Hbuf[l4] = np.concatenate([oh[:, 0, :], oh[:, 1, :]], axis=0).T
        Cbuf[l4] = np.concatenate([oc[:, 0, :], oc[:, 1, :]], axis=0).T

    h0 = _host_top(Hbuf, Cbuf, xs, emb_table, Wx, bx, Wh, bh)
    logits = np.asarray(Wout, dtype=F32) @ h0 + np.asarray(bout, dtype=F32)
    return _log_softmax(logits)



# revision 3
# speedup vs baseline: 1.4524x; 1.4524x over previous
"""ChildSum TreeLSTM (N=8192 nodes, 4-ary static heap tree, H=256, D=300) on 8 trn2 NeuronCores.

Strategy
--------
The tree is static: node i's children are 4i+1..4i+4 (clipped at N). The reverse
scan (children before parents) is equivalent to processing the tree level by
level, bottom-up; nodes within a level are independent, so each level is a
batched LSTM cell (matmuls + elementwise).

Sharding: the 256 level-4 subtrees are partitioned across the 8 cores (balanced
by the number of *internal* level-6 descendants, which determines level-7 leaf
count). Each core processes its forest fully locally — children of a sorted node
range are contiguous in the next level's sorted array, so the recurrence needs
no gathers and no cross-core communication. Cores output their 32 level-4 root
(h, c) states; the tiny top of the tree (levels 3..0, 85 nodes) plus the final
log_softmax run on the host in numpy.

On-device layout: everything is transposed — feature dim on SBUF partitions
(256 features = 2 halves of 128), nodes along the free axis. The child-h sums
and per-child forget gates then become strided slicing along the free axis.
Key perf choices (round 2):
  * bf16 weights + h-state: avoids the f32r 4x small-matmul penalty on the
    recurrence matmuls, halves weight DMA, and unlocks DVE 2x/4x modes.
  * inputs repacked on host to [128, k, cols] so each (queue, priority) chunk
    is ONE large DMA; 9 input DMAs total spread over 4 queues, ordered so the
    first range's operands land first.
  * x-side projections in three uniform 352-column ranges; the forget gate is
    only computed for the 256 internal-node columns (leaves don't use it).
  * leaf-column gate activations are applied directly while draining PSUM
    (act-drain) — raw gx never round-trips through SBUF for leaf columns.
  * biases (bx + bh, zeros in practice) are folded into an extra ones-row of
    the x-side matmul, so pad columns (zero x) self-compute to h = c = 0.
"""

import numpy as np
import ml_dtypes

BF16 = ml_dtypes.bfloat16

N = 8192
H = 256
D = 300
K = 4
OUT = 4
NCORES = 8
L7P = 384           # padded level-7 columns per core (4 * IPMAX)
IPMAX = 96          # max internal level-6 nodes per core
KUSE = 301          # contraction rows actually used (300 emb + 1 ones)
XCOLS = L7P + 512 + 128 + 32   # 1056 per-core node columns: [L7 | L6 | L5 | L4]

GATE_MAP = [0, 2, 3, 1]  # our gate order [i, o, u, f] -> reference gate indices

F32 = np.float32


def _build_plan():
    """Assign the 256 level-4 subtrees to 8 cores; build per-core column maps."""
    # w(u) = number of internal (has-children) level-6 descendants of L4 node u.
    # Full-weight subtrees (w=16) are u in [85, 127); u=127 has w=11; rest 0.
    full = list(range(85, 127))                               # 42 subtrees
    lights = list(range(128, 341))                            # 213 subtrees
    heavy_counts = [6, 6, 5, 5, 5, 5, 5, 5]                   # sums to 42
    light_counts = [26, 26, 26, 27, 27, 27, 27, 27]           # sums to 213
    cores = []
    hpos = 0
    lpos = 0
    for c in range(NCORES):
        hs = full[hpos:hpos + heavy_counts[c]]
        hpos += heavy_counts[c]
        if c == 2:
            hs = hs + [127]                                   # w sums: 96,96,91,80*5
        ls = lights[lpos:lpos + light_counts[c]]
        lpos += light_counts[c]
        cores.append(sorted(hs + ls))
    all_l4 = sorted(u for cs in cores for u in cs)
    assert all_l4 == list(range(85, 341)), "L4 assignment must partition [85, 341)"

    plan = []
    for c in range(NCORES):
        l4 = cores[c]
        assert len(l4) == 32
        l5 = [4 * u + 1 + k for u in l4 for k in range(K)]
        l6 = [4 * v + 1 + k for v in l5 for k in range(K)]
        wc = sum(1 for x in l6 if x < 2048)
        assert wc <= IPMAX
        l7 = []
        for x in l6[:wc]:
            for k in range(K):
                ch = 4 * x + 1 + k
                l7.append(ch if ch < N else -1)
        l7 += [-1] * (L7P - len(l7))
        cols = np.array(l7 + l6 + l5 + l4, dtype=np.int64)
        assert cols.shape == (XCOLS,)
        plan.append((cols, wc, np.array(l4, dtype=np.int64)))
    return plan


_PLAN = _build_plan()

# chunk schedule: (xoff, ncols, ip, child_level, child_col_off, out_level,
# out_off, goff).  goff = column offset of this chunk's internal nodes inside
# the 256-wide internal-gx tiles (GXI / GX3): [L6int 0:96 | L5 96:224 | L4 224:256].
_CHUNKS = [
    (0,    256, 0,   None, 0,   7, 0,   None),   # L7 leaves, part A
    (256,  128, 0,   None, 0,   7, 256, None),   # L7 leaves, part B
    (640,  256, 0,   None, 0,   6, 256, None),   # L6 leaf-only half
    (384,  256, 96,  7,    0,   6, 0,   0),      # L6 internal half
    (896,  128, 128, 6,    0,   5, 0,   96),     # L5
    (1024, 32,  32,  5,    0,   4, 0,   224),    # L4
]
_STATE_COLS = {7: L7P, 6: 512, 5: 128, 4: 32}

# phase-1 column ranges (each <= 512 PSUM f32 columns, uniform for ramping)
_RANGES = [(0, 352), (352, 704), (704, 1056)]
# per range: leaf column windows (act-drained) and internal windows
# (copy-drained to GXI / matmul'd for the f gate), with dst offset in the
# 256-wide internal tiles.
_LEAF_WIN = [[(0, 352)], [(352, 384), (480, 704)], [(704, 896)]]
_INT_WIN = [[], [(384, 480, 0)], [(896, 1056, 96)]]


def _static_tree():
    idx = np.arange(N)[:, None] * K + 1 + np.arange(K)[None, :]
    mask = (idx < N).astype(F32)
    idx = np.where(idx < N, idx, 0).astype(np.int32)
    return idx, mask


_STATIC_IDX, _STATIC_MASK = _static_tree()


def _pack_weights(Wx, bx, Wh, bh):
    """Pack to partition-major HBM layouts.

    wx3  [128, 3, 1024] bf16 : wx3[p, k, 256*g + j] = Wx[rg][j_feat, 128*k + p]
                               row 301 (k=2, p=45) holds bx+bh (ones-row bias).
    wh2  [128, 2, 768]  bf16 : i/o/u recurrence weights, transposed.
    whf2 [128, 2, 256]  bf16 : forget recurrence weights, transposed.
    """
    wx = np.zeros((384, 4 * H), dtype=F32)
    for g, rg in enumerate(GATE_MAP):
        wx[:D, H * g:H * (g + 1)] = np.asarray(Wx[rg], dtype=F32).T
        wx[D, H * g:H * (g + 1)] = np.asarray(bx[rg], dtype=F32) + np.asarray(bh[rg], dtype=F32)
    wx3 = np.ascontiguousarray(wx.reshape(3, 128, 4 * H).transpose(1, 0, 2)).astype(BF16)
    wh = np.zeros((H, 3 * H), dtype=F32)
    for g, rg in enumerate([0, 2, 3]):  # i, o, u
        wh[:, H * g:H * (g + 1)] = np.asarray(Wh[rg], dtype=F32).T
    wh2 = np.ascontiguousarray(wh.reshape(2, 128, 3 * H).transpose(1, 0, 2)).astype(BF16)
    whf = np.asarray(Wh[1], dtype=F32).T
    whf2 = np.ascontiguousarray(whf.reshape(2, 128, H).transpose(1, 0, 2)).astype(BF16)
    return wx3, wh2, whf2


def _pack_xt(xs, emb_table):
    """Per-core transposed embeddings, partition-major: [128, 3, XCOLS] bf16."""
    X = np.asarray(emb_table, dtype=F32)[np.asarray(xs)]
    xts = []
    for cols, _, _ in _PLAN:
        xt = np.zeros((384, XCOLS), dtype=F32)
        real = cols >= 0
        xt[:D, real] = X[cols[real]].T
        xt[D, real] = 1.0
        xt3 = np.ascontiguousarray(xt.reshape(3, 128, XCOLS).transpose(1, 0, 2)).astype(BF16)
        xts.append(xt3)
    return xts


def _sigmoid(x):
    return (1.0 / (1.0 + np.exp(-x))).astype(F32)


def _host_top(Hbuf, Cbuf, xs, emb_table, Wx, bx, Wh, bh):
    """Compute tree levels 3..0 (nodes 0..84) on the host, numpy fp32."""
    Wx = np.asarray(Wx, dtype=F32)
    bx = np.asarray(bx, dtype=F32)
    Wh = np.asarray(Wh, dtype=F32)
    bh = np.asarray(bh, dtype=F32)
    emb = np.asarray(emb_table, dtype=F32)
    xs = np.asarray(xs)
    for lo, hi in [(21, 85), (5, 21), (1, 5), (0, 1)]:
        ids = np.arange(lo, hi)
        Xl = emb[xs[ids]]                                   # [n, D]
        gx = np.einsum('ghd,nd->ngh', Wx, Xl).astype(F32) + bx
        cidx = ids[:, None] * K + 1 + np.arange(K)[None, :]  # all valid (< 341)
        Hc = Hbuf[cidx]
        Cc = Cbuf[cidx]
        hs = Hc.sum(1)
        ig = _sigmoid(gx[:, 0] + hs @ Wh[0].T + bh[0])
        og = _sigmoid(gx[:, 2] + hs @ Wh[2].T + bh[2])
        ug = np.tanh(gx[:, 3] + hs @ Wh[3].T + bh[3]).astype(F32)
        f = _sigmoid(gx[:, 1][:, None, :] + Hc @ Wh[1].T + bh[1])
        cc = ig * ug + (f * Cc).sum(1)
        hh = og * np.tanh(cc).astype(F32)
        Hbuf[ids] = hh
        Cbuf[ids] = cc
    return Hbuf[0]


def _log_softmax(x):
    m = np.max(x)
    e = np.exp(x - m)
    return (x - m - np.log(e.sum())).astype(F32)


# ----------------------------------------------------------------------------
# Bass device program
# ----------------------------------------------------------------------------

_COMPILED = None


def _build_device_program():
    import contextlib

    import concourse.bacc as bacc
    import concourse.tile as tile
    import concourse.mybir as mybir

    f32 = mybir.dt.float32
    bf16 = mybir.dt.bfloat16
    Sig = mybir.ActivationFunctionType.Sigmoid
    Tanh = mybir.ActivationFunctionType.Tanh
    AxX = mybir.AxisListType.X
    Add = mybir.AluOpType.add

    nc = bacc.Bacc("TRN2", target_bir_lowering=False, debug=False,
                   num_devices=NCORES)

    xt_d = nc.dram_tensor("xt", [128, 3, XCOLS], bf16, kind="ExternalInput")
    wx_d = nc.dram_tensor("wx", [128, 3, 4 * H], bf16, kind="ExternalInput")
    wh_d = nc.dram_tensor("wh", [128, 2, 3 * H], bf16, kind="ExternalInput")
    whf_d = nc.dram_tensor("whf", [128, 2, H], bf16, kind="ExternalInput")
    out_h_d = nc.dram_tensor("out_h", [128, 2, 32], bf16, kind="ExternalOutput")
    out_c_d = nc.dram_tensor("out_c", [128, 2, 32], f32, kind="ExternalOutput")

    GFUNC = [Sig, Sig, Tanh]   # activation per gate i, o, u

    with tile.TileContext(nc) as tc:
        with contextlib.ExitStack() as ctx:
            inp = ctx.enter_context(tc.tile_pool(name="inp", bufs=1))
            st = ctx.enter_context(tc.tile_pool(name="state", bufs=1))
            wk = ctx.enter_context(tc.tile_pool(name="work", bufs=2))
            ps = ctx.enter_context(
                tc.tile_pool(name="psum", bufs=1, space="PSUM"))

            # --- input SBUF tiles (single tile per tensor; sliced DMAs)
            xt_s = inp.tile([128, 3, XCOLS], bf16, tag="xt", name="xt")
            wx_s = inp.tile([128, 3, 4 * H], bf16, tag="wx", name="wx")
            wh_s = inp.tile([128, 2, 3 * H], bf16, tag="wh", name="wh")
            whf_s = inp.tile([128, 2, H], bf16, tag="whf", name="whf")

            # priority DMAs (only sync / scalar / gpsimd queues can issue
            # DMAs): range 0 split by k across the three queues, with the
            # first weight quarter leading on scalar.
            a0, b0 = _RANGES[0]
            nc.scalar.dma_start(out=wx_s[:, :, 0:256], in_=wx_d[:, :, 0:256])
            nc.sync.dma_start(out=xt_s[:, 0, a0:b0], in_=xt_d[:, 0, a0:b0])
            nc.scalar.dma_start(out=xt_s[:, 1, a0:b0], in_=xt_d[:, 1, a0:b0])
            nc.gpsimd.dma_start(out=xt_s[:, 2, a0:b0], in_=xt_d[:, 2, a0:b0])
            # remaining ranges / quarters, less urgent
            a1, b1 = _RANGES[1]
            a2, b2 = _RANGES[2]
            nc.sync.dma_start(out=xt_s[:, :, a1:b1], in_=xt_d[:, :, a1:b1])
            nc.scalar.dma_start(out=xt_s[:, :, a2:b2], in_=xt_d[:, :, a2:b2])
            for q in range(1, 4):
                nc.gpsimd.dma_start(out=wx_s[:, :, 256 * q:256 * (q + 1)],
                                    in_=wx_d[:, :, 256 * q:256 * (q + 1)])
            nc.gpsimd.dma_start(out=wh_s[:], in_=wh_d[:])
            nc.gpsimd.dma_start(out=whf_s[:], in_=whf_d[:])

            # --- persistent state + gate tiles
            SH = {lv: st.tile([128, 2, n], bf16, tag=f"h{lv}", name=f"sh{lv}")
                  for lv, n in _STATE_COLS.items()}
            SC = {lv: st.tile([128, 2, n], f32, tag=f"c{lv}", name=f"sc{lv}")
                  for lv, n in _STATE_COLS.items()}
            # activated gates for ALL columns (leaf cols filled by act-drain
            # in phase 1; internal cols filled during phase 2)
            G = [st.tile([128, 2, XCOLS], bf16, tag=f"g{g}", name=f"g{g}")
                 for g in range(3)]
            # raw gx for internal columns only (i/o/u for the psum add; f for
            # the per-child forget bias)
            GXI = [st.tile([128, 2, 256], f32, tag=f"gxi{g}", name=f"gxi{g}")
                   for g in range(3)]
            GX3 = st.tile([128, 2, 256], f32, tag="gx3", name="gx3")

            with nc.allow_low_precision("bf16 gates/h-state within 2e-2 tol"):
                # --- phase 1: x-side projections, range-major sweep.
                # leaf windows act-drained straight into G; internal windows
                # copy-drained into GXI; f-gate computed only on internal
                # windows into a single psum tile, drained to GX3.
                gxf = ps.tile([128, 2, 256], f32, tag="gxf", name="gxf")
                for ri, (a, b) in enumerate(_RANGES):
                    for m in range(6):
                        g, phi = divmod(m, 2)
                        col = 128 * m
                        pt = ps.tile([128, 512], f32, tag="gx", bufs=3,
                                     name=f"pgx{ri}_{m}")
                        for k in range(3):
                            nc.tensor.matmul(
                                pt[:, 0:b - a],
                                wx_s[:, k, col:col + 128],
                                xt_s[:, k, a:b],
                                start=(k == 0), stop=(k == 2))
                        for (wa, wb) in _LEAF_WIN[ri]:
                            nc.scalar.activation(
                                G[g][:, phi, wa:wb], pt[:, wa - a:wb - a],
                                GFUNC[g])
                        for (wa, wb, go) in _INT_WIN[ri]:
                            nc.vector.tensor_copy(
                                GXI[g][:, phi, go:go + wb - wa],
                                pt[:, wa - a:wb - a])
                    for (wa, wb, go) in _INT_WIN[ri]:
                        for phi in range(2):
                            col = 768 + 128 * phi
                            for k in range(3):
                                nc.tensor.matmul(
                                    gxf[:, phi, go:go + wb - wa],
                                    wx_s[:, k, col:col + 128],
                                    xt_s[:, k, wa:wb],
                                    start=(k == 0), stop=(k == 2))
                nc.vector.tensor_copy(GX3[:], gxf[:])

                # --- phase 2: levels bottom-up in chunks
                for (xoff, cn, ip, child, coff, outlv, ooff, goff) in _CHUNKS:
                    if ip > 0:
                        # hs = sum of the 4 child h columns per node
                        hs = wk.tile([128, 2, ip], bf16, tag="hs", name="hs")
                        cv = SH[child][:, :, coff:coff + 4 * ip].rearrange(
                            "p t (n k) -> p t n k", k=K)
                        nc.vector.tensor_reduce(hs[:], cv, axis=AxX, op=Add)

                        # i/o/u gates: wh @ hs accumulated on top of gx
                        for g in range(3):
                            P = ps.tile([128, 2, 128], f32, tag="pa", bufs=2,
                                        name=f"pa{outlv}_{g}")
                            for phi in range(2):
                                for k in range(2):
                                    nc.tensor.matmul(
                                        P[:, phi, 0:ip],
                                        wh_s[:, k, 256 * g + 128 * phi:
                                             256 * g + 128 * phi + 128],
                                        hs[:, k, 0:ip],
                                        start=(k == 0), stop=(k == 1))
                            nc.vector.tensor_add(
                                P[:, :, 0:ip], P[:, :, 0:ip],
                                GXI[g][:, :, goff:goff + ip])
                            nc.scalar.activation(
                                G[g][:, :, xoff:xoff + ip], P[:, :, 0:ip],
                                GFUNC[g])

                        # forget: f = sigmoid(gf + Whf @ h_child) per child;
                        # csum = sum_k f_k * c_child_k
                        Pfc = ps.tile([128, 2, 512], f32, tag="pf", name="pfc")
                        for phi in range(2):
                            for k in range(2):
                                nc.tensor.matmul(
                                    Pfc[:, phi, 0:4 * ip],
                                    whf_s[:, k, 128 * phi:128 * phi + 128],
                                    SH[child][:, k, coff:coff + 4 * ip],
                                    start=(k == 0), stop=(k == 1))
                        pv = Pfc[:, :, 0:4 * ip].rearrange(
                            "p t (n k) -> p t n k", k=K)
                        gfb = GX3[:, :, goff:goff + ip][:, :, :, None]
                        nc.vector.tensor_add(
                            pv, pv, gfb.broadcast_to([128, 2, ip, K]))
                        FS = wk.tile([128, 2, 512], f32, tag="fs", name="fs")
                        nc.scalar.activation(
                            FS[:, :, 0:4 * ip], Pfc[:, :, 0:4 * ip], Sig)
                        nc.gpsimd.tensor_mul(
                            FS[:, :, 0:4 * ip], FS[:, :, 0:4 * ip],
                            SC[child][:, :, coff:coff + 4 * ip])
                        sv = FS[:, :, 0:4 * ip].rearrange(
                            "p t (n k) -> p t n k", k=K)
                        csum = wk.tile([128, 2, ip], f32, tag="csum",
                                       name="csum")
                        nc.gpsimd.tensor_add(csum[:], sv[:, :, :, 0],
                                             sv[:, :, :, 1])
                        nc.gpsimd.tensor_add(csum[:], csum[:], sv[:, :, :, 2])
                        nc.gpsimd.tensor_add(csum[:], csum[:], sv[:, :, :, 3])

                    # c = ig*ug (+ csum on internal cols); h = og*tanh(c)
                    Cdst = SC[outlv][:, :, ooff:ooff + cn]
                    nc.vector.tensor_mul(
                        Cdst, G[0][:, :, xoff:xoff + cn],
                        G[2][:, :, xoff:xoff + cn])
                    if ip > 0:
                        nc.vector.tensor_add(
                            SC[outlv][:, :, ooff:ooff + ip],
                            SC[outlv][:, :, ooff:ooff + ip],
                            csum[:])
                    TC = wk.tile([128, 2, 256], bf16, tag="tc", name="tc")
                    nc.scalar.activation(TC[:, :, 0:cn], Cdst, Tanh)
                    nc.vector.tensor_mul(
                        SH[outlv][:, :, ooff:ooff + cn],
                        G[1][:, :, xoff:xoff + cn], TC[:, :, 0:cn])

            nc.sync.dma_start(out=out_h_d[:], in_=SH[4][:])
            nc.sync.dma_start(out=out_c_d[:], in_=SC[4][:])

    nc.compile()
    return nc


def _get_compiled():
    global _COMPILED
    if _COMPILED is None:
        _COMPILED = _build_device_program()
    return _COMPILED


def _numpy_fallback(xs, child_idx, child_mask, emb_table, Wx, bx, Wh, bh,
                    Wout, bout):
    """Exact sequential scan in numpy; only used if the tree is not the
    expected static 4-ary heap."""
    X = np.asarray(emb_table, dtype=F32)[np.asarray(xs)]
    Wx = np.asarray(Wx, dtype=F32)
    Wh = np.asarray(Wh, dtype=F32)
    bx = np.asarray(bx, dtype=F32)
    bh = np.asarray(bh, dtype=F32)
    gx = np.einsum('ghd,nd->ngh', Wx, X).astype(F32) + bx
    Hb = np.zeros((N, H), dtype=F32)
    Cb = np.zeros((N, H), dtype=F32)
    ci = np.asarray(child_idx)
    cm = np.asarray(child_mask, dtype=F32)
    for i in range(N - 1, -1, -1):
        idx = ci[i]
        m = cm[i][:, None]
        Hc = Hb[idx] * m
        Cc = Cb[idx] * m
        hs = Hc.sum(0)
        g = gx[i]
        ig = _sigmoid(g[0] + Wh[0] @ hs + bh[0])
        og = _sigmoid(g[2] + Wh[2] @ hs + bh[2])
        ug = np.tanh(g[3] + Wh[3] @ hs + bh[3]).astype(F32)
        f = _sigmoid(g[1] + Hc @ Wh[1].T + bh[1])
        c = ig * ug + (f * Cc).sum(0)
        Hb[i] = og * np.tanh(c).astype(F32)
        Cb[i] = c
    logits = np.asarray(Wout, dtype=F32) @ Hb[0] + np.asarray(bout, dtype=F32)
    return _log_softmax(logits)


def kernel(xs, child_idx, child_mask, emb_table, Wx, bx, Wh, bh, Wout, bout):
    xs = np.asarray(xs)
    if not (np.array_equal(np.asarray(child_idx), _STATIC_IDX)
            and np.array_equal(np.asarray(child_mask, dtype=F32), _STATIC_MASK)):
        return _numpy_fallback(xs, child_idx, child_mask, emb_table, Wx, bx,
                               Wh, bh, Wout, bout)

    from concourse.bass_utils import run_bass_kernel_spmd

    wx3, wh2, whf2 = _pack_weights(Wx, bx, Wh, bh)
    xts = _pack_xt(xs, emb_table)
    in_maps = [
        {"xt": xts[c], "wx": wx3, "wh": wh2, "whf": whf2}
        for c in range(NCORES)
    ]
    nc = _get_compiled()
    res = run_bass_kernel_spmd(nc, in_maps, core_ids=list(range(NCORES)))

    Hbuf = np.zeros((341, H), dtype=F32)
    Cbuf = np.zeros((341, H), dtype=F32)
    for c in range(NCORES):
        _, _, l4 = _PLAN[c]
        oh = np.asarray(res.results[c]["out_h"], dtype=F32)   # [128, 2, 32]
        oc = res.results[c]["out_c"]
        Hbuf[l4] = np.concatenate([oh[:, 0, :], oh[:, 1, :]], axis=0).T
        Cbuf[l4] = np.concatenate([oc[:, 0, :], oc[:, 1, :]], axis=0).T

    h0 = _host_top(Hbuf, Cbuf, xs, emb_table, Wx, bx, Wh, bh)
    logits = np.asarray(Wout, dtype=F32) @ h0 + np.asarray(bout, dtype=F32)
    return _log_softmax(logits)


# revision 10
# speedup vs baseline: 1.6095x; 1.1081x over previous
"""ChildSum TreeLSTM (N=8192 nodes, 4-ary static heap tree, H=256, D=300) on 8 trn2 NeuronCores.

Strategy
--------
The tree is static: node i's children are 4i+1..4i+4 (clipped at N). The reverse
scan (children before parents) is equivalent to processing the tree level by
level, bottom-up; nodes within a level are independent, so each level is a
batched LSTM cell (matmuls + elementwise).

Sharding: the 256 level-4 subtrees are partitioned across the 8 cores (balanced
by the number of *internal* level-6 descendants, which determines level-7 leaf
count). Each core processes its forest fully locally — children of a sorted node
range are contiguous in the next level's sorted array, so the recurrence needs
no gathers and no cross-core communication. Cores output their 32 level-4 root
(h, c) states; the tiny top of the tree (levels 3..0, 85 nodes) plus the final
log_softmax run on the host in numpy.

On-device layout: everything is transposed — feature dim on SBUF partitions
(256 features = 2 halves of 128), nodes along the free axis. The child-h sums
and per-child forget gates then become strided slicing along the free axis.
Key perf choices (round 2):
  * bf16 weights + h-state: avoids the f32r 4x small-matmul penalty on the
    recurrence matmuls, halves weight DMA, and unlocks DVE 2x/4x modes.
  * inputs repacked on host to [128, k, cols] so each (queue, priority) chunk
    is ONE large DMA; 9 input DMAs total spread over 4 queues, ordered so the
    first range's operands land first.
  * x-side projections in three uniform 352-column ranges; the forget gate is
    only computed for the 256 internal-node columns (leaves don't use it).
  * leaf-column gate activations are applied directly while draining PSUM
    (act-drain) — raw gx never round-trips through SBUF for leaf columns.
  * biases (bx + bh, zeros in practice) are folded into an extra ones-row of
    the x-side matmul, so pad columns (zero x) self-compute to h = c = 0.
"""

import numpy as np
import ml_dtypes

BF16 = ml_dtypes.bfloat16

N = 8192
H = 256
D = 300
K = 4
OUT = 4
NCORES = 8
L7P = 384           # padded level-7 columns per core (4 * IPMAX)
IPMAX = 96          # max internal level-6 nodes per core
KUSE = 301          # contraction rows actually used (300 emb + 1 ones)
XCOLS = L7P + 512 + 128 + 32   # 1056 per-core node columns: [L7 | L6 | L5 | L4]

GATE_MAP = [0, 2, 3, 1]  # our gate order [i, o, u, f] -> reference gate indices

F32 = np.float32


def _build_plan():
    """Assign the 256 level-4 subtrees to 8 cores; build per-core column maps."""
    # w(u) = number of internal (has-children) level-6 descendants of L4 node u.
    # Full-weight subtrees (w=16) are u in [85, 127); u=127 has w=11; rest 0.
    full = list(range(85, 127))                               # 42 subtrees
    lights = list(range(128, 341))                            # 213 subtrees
    heavy_counts = [6, 6, 5, 5, 5, 5, 5, 5]                   # sums to 42
    light_counts = [26, 26, 26, 27, 27, 27, 27, 27]           # sums to 213
    cores = []
    hpos = 0
    lpos = 0
    for c in range(NCORES):
        hs = full[hpos:hpos + heavy_counts[c]]
        hpos += heavy_counts[c]
        if c == 2:
            hs = hs + [127]                                   # w sums: 96,96,91,80*5
        ls = lights[lpos:lpos + light_counts[c]]
        lpos += light_counts[c]
        cores.append(sorted(hs + ls))
    all_l4 = sorted(u for cs in cores for u in cs)
    assert all_l4 == list(range(85, 341)), "L4 assignment must partition [85, 341)"

    plan = []
    for c in range(NCORES):
        l4 = cores[c]
        assert len(l4) == 32
        l5 = [4 * u + 1 + k for u in l4 for k in range(K)]
        l6 = [4 * v + 1 + k for v in l5 for k in range(K)]
        wc = sum(1 for x in l6 if x < 2048)
        assert wc <= IPMAX
        l7 = []
        for x in l6[:wc]:
            for k in range(K):
                ch = 4 * x + 1 + k
                l7.append(ch if ch < N else -1)
        l7 += [-1] * (L7P - len(l7))
        cols = np.array(l7 + l6 + l5 + l4, dtype=np.int64)
        assert cols.shape == (XCOLS,)
        plan.append((cols, wc, np.array(l4, dtype=np.int64)))
    return plan


_PLAN = _build_plan()

# chunk schedule: (xoff, ncols, ip, child_level, child_col_off, out_level,
# out_off, goff).  goff = column offset of this chunk's internal nodes inside
# the 256-wide internal-gx tiles (GXI / GX3): [L6int 0:96 | L5 96:224 | L4 224:256].
_CHUNKS = [
    (0,    256, 0,   None, 0,   7, 0,   None),   # L7 leaves, part A
    (256,  128, 0,   None, 0,   7, 256, None),   # L7 leaves, part B
    (640,  256, 0,   None, 0,   6, 256, None),   # L6 leaf-only half
    (384,  256, 96,  7,    0,   6, 0,   0),      # L6 internal half
    (896,  128, 128, 6,    0,   5, 0,   96),     # L5
    (1024, 32,  32,  5,    0,   4, 0,   224),    # L4
]
_STATE_COLS = {7: L7P, 6: 512, 5: 128, 4: 32}

# phase-1 column ranges (each <= 512 PSUM f32 columns, uniform for ramping)
_RANGES = [(0, 352), (352, 704), (704, 1056)]
# per range: leaf column windows (act-drained) and internal windows
# (copy-drained to GXI / matmul'd for the f gate), with dst offset in the
# 256-wide internal tiles.
_LEAF_WIN = [[(0, 352)], [(352, 384), (480, 704)], [(704, 896)]]
_INT_WIN = [[], [(384, 480, 0)], [(896, 1056, 96)]]


def _static_tree():
    idx = np.arange(N)[:, None] * K + 1 + np.arange(K)[None, :]
    mask = (idx < N).astype(F32)
    idx = np.where(idx < N, idx, 0).astype(np.int32)
    return idx, mask


_STATIC_IDX, _STATIC_MASK = _static_tree()


def _pack_weights(Wx, bx, Wh, bh):
    """Pack to partition-major HBM layouts.

    wx3  [128, 3, 1024] bf16 : wx3[p, k, 256*g + j] = Wx[rg][j_feat, 128*k + p]
                               row 301 (k=2, p=45) holds bx+bh (ones-row bias).
    wh2  [128, 2, 768]  bf16 : i/o/u recurrence weights, transposed.
    whf2 [128, 2, 256]  bf16 : forget recurrence weights, transposed.
    """
    wx = np.zeros((384, 4 * H), dtype=F32)
    for g, rg in enumerate(GATE_MAP):
        wx[:D, H * g:H * (g + 1)] = np.asarray(Wx[rg], dtype=F32).T
        wx[D, H * g:H * (g + 1)] = np.asarray(bx[rg], dtype=F32) + np.asarray(bh[rg], dtype=F32)
    wx3 = np.ascontiguousarray(wx.reshape(3, 128, 4 * H).transpose(1, 0, 2)).astype(BF16)
    wh = np.zeros((H, 3 * H), dtype=F32)
    for g, rg in enumerate([0, 2, 3]):  # i, o, u
        wh[:, H * g:H * (g + 1)] = np.asarray(Wh[rg], dtype=F32).T
    wh2 = np.ascontiguousarray(wh.reshape(2, 128, 3 * H).transpose(1, 0, 2)).astype(BF16)
    whf = np.asarray(Wh[1], dtype=F32).T
    whf2 = np.ascontiguousarray(whf.reshape(2, 128, H).transpose(1, 0, 2)).astype(BF16)
    return wx3, wh2, whf2


def _pack_xt(xs, emb_table):
    """Per-core transposed embeddings, partition-major: [128, 3, XCOLS] bf16."""
    X = np.asarray(emb_table, dtype=F32)[np.asarray(xs)]
    xts = []
    for cols, _, _ in _PLAN:
        xt = np.zeros((384, XCOLS), dtype=F32)
        real = cols >= 0
        xt[:D, real] = X[cols[real]].T
        xt[D, real] = 1.0
        xt3 = np.ascontiguousarray(xt.reshape(3, 128, XCOLS).transpose(1, 0, 2)).astype(BF16)
        xts.append(xt3)
    return xts


def _sigmoid(x):
    return (1.0 / (1.0 + np.exp(-x))).astype(F32)


def _host_top(Hbuf, Cbuf, xs, emb_table, Wx, bx, Wh, bh):
    """Compute tree levels 3..0 (nodes 0..84) on the host, numpy fp32."""
    Wx = np.asarray(Wx, dtype=F32)
    bx = np.asarray(bx, dtype=F32)
    Wh = np.asarray(Wh, dtype=F32)
    bh = np.asarray(bh, dtype=F32)
    emb = np.asarray(emb_table, dtype=F32)
    xs = np.asarray(xs)
    for lo, hi in [(21, 85), (5, 21), (1, 5), (0, 1)]:
        ids = np.arange(lo, hi)
        Xl = emb[xs[ids]]                                   # [n, D]
        gx = np.einsum('ghd,nd->ngh', Wx, Xl).astype(F32) + bx
        cidx = ids[:, None] * K + 1 + np.arange(K)[None, :]  # all valid (< 341)
        Hc = Hbuf[cidx]
        Cc = Cbuf[cidx]
        hs = Hc.sum(1)
        ig = _sigmoid(gx[:, 0] + hs @ Wh[0].T + bh[0])
        og = _sigmoid(gx[:, 2] + hs @ Wh[2].T + bh[2])
        ug = np.tanh(gx[:, 3] + hs @ Wh[3].T + bh[3]).astype(F32)
        f = _sigmoid(gx[:, 1][:, None, :] + Hc @ Wh[1].T + bh[1])
        cc = ig * ug + (f * Cc).sum(1)
        hh = og * np.tanh(cc).astype(F32)
        Hbuf[ids] = hh
        Cbuf[ids] = cc
    return Hbuf[0]


def _log_softmax(x):
    m = np.max(x)
    e = np.exp(x - m)
    return (x - m - np.log(e.sum())).astype(F32)


# ----------------------------------------------------------------------------
# Bass device program
# ----------------------------------------------------------------------------

_COMPILED = None


def _build_device_program():
    import contextlib

    import concourse.bacc as bacc
    import concourse.tile as tile
    import concourse.mybir as mybir

    f32 = mybir.dt.float32
    bf16 = mybir.dt.bfloat16
    Sig = mybir.ActivationFunctionType.Sigmoid
    Tanh = mybir.ActivationFunctionType.Tanh
    AxX = mybir.AxisListType.X
    Add = mybir.AluOpType.add

    nc = bacc.Bacc("TRN2", target_bir_lowering=False, debug=False,
                   num_devices=NCORES)

    xt_d = nc.dram_tensor("xt", [128, 3, XCOLS], bf16, kind="ExternalInput")
    wx_d = nc.dram_tensor("wx", [128, 3, 4 * H], bf16, kind="ExternalInput")
    wh_d = nc.dram_tensor("wh", [128, 2, 3 * H], bf16, kind="ExternalInput")
    whf_d = nc.dram_tensor("whf", [128, 2, H], bf16, kind="ExternalInput")
    out_h_d = nc.dram_tensor("out_h", [128, 2, 32], bf16, kind="ExternalOutput")
    out_c_d = nc.dram_tensor("out_c", [128, 2, 32], bf16, kind="ExternalOutput")

    GFUNC = [Sig, Sig, Tanh]   # activation per gate i, o, u

    with tile.TileContext(nc) as tc:
        with contextlib.ExitStack() as ctx:
            inp = ctx.enter_context(tc.tile_pool(name="inp", bufs=1))
            st = ctx.enter_context(tc.tile_pool(name="state", bufs=1))
            wk = ctx.enter_context(tc.tile_pool(name="work", bufs=2))
            ps = ctx.enter_context(
                tc.tile_pool(name="psum", bufs=1, space="PSUM"))

            # --- input SBUF tiles (single tile per tensor; sliced DMAs)
            xt_s = inp.tile([128, 3, XCOLS], bf16, tag="xt", name="xt")
            wx_s = inp.tile([128, 3, 4 * H], bf16, tag="wx", name="wx")
            wh_s = inp.tile([128, 2, 3 * H], bf16, tag="wh", name="wh")
            whf_s = inp.tile([128, 2, H], bf16, tag="whf", name="whf")

            # priority DMAs (only sync / scalar / gpsimd queues can issue
            # DMAs): range 0 split by k across the three queues, with the
            # first weight quarter leading on scalar.
            a0, b0 = _RANGES[0]
            nc.scalar.dma_start(out=wx_s[:, :, 0:256], in_=wx_d[:, :, 0:256])
            nc.sync.dma_start(out=xt_s[:, 0, a0:b0], in_=xt_d[:, 0, a0:b0])
            nc.scalar.dma_start(out=xt_s[:, 1, a0:b0], in_=xt_d[:, 1, a0:b0])
            nc.gpsimd.dma_start(out=xt_s[:, 2, a0:b0], in_=xt_d[:, 2, a0:b0])
            # remaining ranges / quarters, less urgent
            a1, b1 = _RANGES[1]
            a2, b2 = _RANGES[2]
            nc.sync.dma_start(out=xt_s[:, :, a1:b1], in_=xt_d[:, :, a1:b1])
            nc.scalar.dma_start(out=xt_s[:, :, a2:b2], in_=xt_d[:, :, a2:b2])
            for q in range(1, 4):
                nc.gpsimd.dma_start(out=wx_s[:, :, 256 * q:256 * (q + 1)],
                                    in_=wx_d[:, :, 256 * q:256 * (q + 1)])
            nc.sync.dma_start(out=wh_s[:], in_=wh_d[:])
            nc.sync.dma_start(out=whf_s[:], in_=whf_d[:])

            # --- persistent state + gate tiles
            SH = {lv: st.tile([128, 2, n], bf16, tag=f"h{lv}", name=f"sh{lv}")
                  for lv, n in _STATE_COLS.items()}
            SC = {lv: st.tile([128, 2, n], bf16, tag=f"c{lv}", name=f"sc{lv}")
                  for lv, n in _STATE_COLS.items()}
            # activated gates for ALL columns (leaf cols filled by act-drain
            # in phase 1; internal cols filled during phase 2)
            G = [st.tile([128, 2, XCOLS], bf16, tag=f"g{g}", name=f"g{g}")
                 for g in range(3)]
            # raw gx for internal columns only (i/o/u for the psum add; f for
            # the per-child forget bias)
            GXI = [st.tile([128, 2, 256], f32, tag=f"gxi{g}", name=f"gxi{g}")
                   for g in range(3)]
            GX3 = st.tile([128, 2, 256], f32, tag="gx3", name="gx3")

            with nc.allow_low_precision("bf16 gates/h-state within 2e-2 tol"):
                # --- phase 1: x-side projections, range-major sweep.
                # leaf windows act-drained straight into G; internal windows
                # copy-drained into GXI; f-gate computed only on internal
                # windows into a single psum tile, drained to GX3.
                gxf = ps.tile([128, 2, 256], f32, tag="gxf", name="gxf")
                for ri, (a, b) in enumerate(_RANGES):
                    for m in range(6):
                        g, phi = divmod(m, 2)
                        col = 128 * m
                        pt = ps.tile([128, 512], f32, tag="gx", bufs=3,
                                     name=f"pgx{ri}_{m}")
                        for k in range(3):
                            nc.tensor.matmul(
                                pt[:, 0:b - a],
                                wx_s[:, k, col:col + 128],
                                xt_s[:, k, a:b],
                                start=(k == 0), stop=(k == 2))
                        # act-drain the whole range into G (internal cols are
                        # overwritten by phase 2); raw gx for internal cols is
                        # copy-drained separately.
                        nc.scalar.activation(
                            G[g][:, phi, a:b], pt[:, 0:b - a], GFUNC[g])
                        for (wa, wb, go) in _INT_WIN[ri]:
                            nc.vector.tensor_copy(
                                GXI[g][:, phi, go:go + wb - wa],
                                pt[:, wa - a:wb - a])
                    for (wa, wb, go) in _INT_WIN[ri]:
                        for phi in range(2):
                            col = 768 + 128 * phi
                            for k in range(3):
                                nc.tensor.matmul(
                                    gxf[:, phi, go:go + wb - wa],
                                    wx_s[:, k, col:col + 128],
                                    xt_s[:, k, wa:wb],
                                    start=(k == 0), stop=(k == 2))
                nc.vector.tensor_copy(GX3[:], gxf[:])

                # --- phase 2: levels bottom-up in chunks
                for (xoff, cn, ip, child, coff, outlv, ooff, goff) in _CHUNKS:
                    if ip > 0:
                        # hs = sum of the 4 child h columns per node
                        hs = wk.tile([128, 2, ip], bf16, tag="hs", name="hs")
                        cv = SH[child][:, :, coff:coff + 4 * ip].rearrange(
                            "p t (n k) -> p t n k", k=K)
                        nc.vector.tensor_reduce(hs[:], cv, axis=AxX, op=Add)

                        # i/o/u gates: preload gx into PSUM (off the critical
                        # chain), then accumulate wh @ hs on top of it.
                        Ps = []
                        for g in range(3):
                            P = ps.tile([128, 2, 128], f32, tag="pa", bufs=2,
                                        name=f"pa{outlv}_{g}")
                            nc.vector.tensor_copy(
                                P[:, :, 0:ip], GXI[g][:, :, goff:goff + ip])
                            Ps.append(P)
                        # forget psum: preload the per-child gf broadcast
                        Pfc = ps.tile([128, 2, 512], f32, tag="pf", name="pfc")
                        pv = Pfc[:, :, 0:4 * ip].rearrange(
                            "p t (n k) -> p t n k", k=K)
                        gfb = GX3[:, :, goff:goff + ip][:, :, :, None]
                        nc.vector.tensor_copy(
                            pv, gfb.broadcast_to([128, 2, ip, K]))

                        for g in range(3):
                            P = Ps[g]
                            for phi in range(2):
                                for k in range(2):
                                    nc.tensor.matmul(
                                        P[:, phi, 0:ip],
                                        wh_s[:, k, 256 * g + 128 * phi:
                                             256 * g + 128 * phi + 128],
                                        hs[:, k, 0:ip],
                                        start=False, stop=(k == 1),
                                        skip_group_check=True)
                            nc.scalar.activation(
                                G[g][:, :, xoff:xoff + ip], P[:, :, 0:ip],
                                GFUNC[g])

                        # f = sigmoid(gf + Whf @ h_child) per child;
                        # csum = sum_k f_k * c_child_k
                        FS = wk.tile([128, 2, 512], bf16, tag="fs", name="fs")
                        for phi in range(2):
                            for k in range(2):
                                nc.tensor.matmul(
                                    Pfc[:, phi, 0:4 * ip],
                                    whf_s[:, k, 128 * phi:128 * phi + 128],
                                    SH[child][:, k, coff:coff + 4 * ip],
                                    start=False, stop=(k == 1),
                                    skip_group_check=True)
                            nc.scalar.activation(
                                FS[:, phi, 0:4 * ip], Pfc[:, phi, 0:4 * ip],
                                Sig)
                        nc.vector.tensor_mul(
                            FS[:, :, 0:4 * ip], FS[:, :, 0:4 * ip],
                            SC[child][:, :, coff:coff + 4 * ip])
                        sv = FS[:, :, 0:4 * ip].rearrange(
                            "p t (n k) -> p t n k", k=K)
                        csum = wk.tile([128, 2, ip], bf16, tag="csum",
                                       name="csum")
                        nc.vector.tensor_reduce(csum[:], sv, axis=AxX, op=Add)

                    # c = ig*ug (+ csum on internal cols); h = og*tanh(c)
                    Cdst = SC[outlv][:, :, ooff:ooff + cn]
                    nc.vector.tensor_mul(
                        Cdst, G[0][:, :, xoff:xoff + cn],
                        G[2][:, :, xoff:xoff + cn])
                    if ip > 0:
                        nc.vector.tensor_add(
                            SC[outlv][:, :, ooff:ooff + ip],
                            SC[outlv][:, :, ooff:ooff + ip],
                            csum[:])
                    TC = wk.tile([128, 2, 256], bf16, tag="tc", name="tc")
                    nc.scalar.activation(TC[:, :, 0:cn], Cdst, Tanh)
                    nc.vector.tensor_mul(
                        SH[outlv][:, :, ooff:ooff + cn],
                        G[1][:, :, xoff:xoff + cn], TC[:, :, 0:cn])

            nc.sync.dma_start(out=out_h_d[:], in_=SH[4][:])
            nc.sync.dma_start(out=out_c_d[:], in_=SC[4][:])

    nc.compile()
    return nc


def _get_compiled():
    global _COMPILED
    if _COMPILED is None:
        _COMPILED = _build_device_program()
    return _COMPILED


def _numpy_fallback(xs, child_idx, child_mask, emb_table, Wx, bx, Wh, bh,
                    Wout, bout):
    """Exact sequential scan in numpy; only used if the tree is not the
    expected static 4-ary heap."""
    X = np.asarray(emb_table, dtype=F32)[np.asarray(xs)]
    Wx = np.asarray(Wx, dtype=F32)
    Wh = np.asarray(Wh, dtype=F32)
    bx = np.asarray(bx, dtype=F32)
    bh = np.asarray(bh, dtype=F32)
    gx = np.einsum('ghd,nd->ngh', Wx, X).astype(F32) + bx
    Hb = np.zeros((N, H), dtype=F32)
    Cb = np.zeros((N, H), dtype=F32)
    ci = np.asarray(child_idx)
    cm = np.asarray(child_mask, dtype=F32)
    for i in range(N - 1, -1, -1):
        idx = ci[i]
        m = cm[i][:, None]
        Hc = Hb[idx] * m
        Cc = Cb[idx] * m
        hs = Hc.sum(0)
        g = gx[i]
        ig = _sigmoid(g[0] + Wh[0] @ hs + bh[0])
        og = _sigmoid(g[2] + Wh[2] @ hs + bh[2])
        ug = np.tanh(g[3] + Wh[3] @ hs + bh[3]).astype(F32)
        f = _sigmoid(g[1] + Hc @ Wh[1].T + bh[1])
        c = ig * ug + (f * Cc).sum(0)
        Hb[i] = og * np.tanh(c).astype(F32)
        Cb[i] = c
    logits = np.asarray(Wout, dtype=F32) @ Hb[0] + np.asarray(bout, dtype=F32)
    return _log_softmax(logits)


def kernel(xs, child_idx, child_mask, emb_table, Wx, bx, Wh, bh, Wout, bout):
    xs = np.asarray(xs)
    if not (np.array_equal(np.asarray(child_idx), _STATIC_IDX)
            and np.array_equal(np.asarray(child_mask, dtype=F32), _STATIC_MASK)):
        return _numpy_fallback(xs, child_idx, child_mask, emb_table, Wx, bx,
                               Wh, bh, Wout, bout)

    from concourse.bass_utils import run_bass_kernel_spmd

    wx3, wh2, whf2 = _pack_weights(Wx, bx, Wh, bh)
    xts = _pack_xt(xs, emb_table)
    in_maps = [
        {"xt": xts[c], "wx": wx3, "wh": wh2, "whf": whf2}
        for c in range(NCORES)
    ]
    nc = _get_compiled()
    res = run_bass_kernel_spmd(nc, in_maps, core_ids=list(range(NCORES)))

    Hbuf = np.zeros((341, H), dtype=F32)
    Cbuf = np.zeros((341, H), dtype=F32)
    for c in range(NCORES):
        _, _, l4 = _PLAN[c]
        oh = np.asarray(res.results[c]["out_h"], dtype=F32)   # [128, 2, 32]
        oc = np.asarray(res.results[c]["out_c"], dtype=F32)
        Hbuf[l4] = np.concatenate([oh[:, 0, :], oh[:, 1, :]], axis=0).T
        Cbuf[l4] = np.concatenate([oc[:, 0, :], oc[:, 1, :]], axis=0).T

    h0 = _host_top(Hbuf, Cbuf, xs, emb_table, Wx, bx, Wh, bh)
    logits = np.asarray(Wout, dtype=F32) @ h0 + np.asarray(bout, dtype=F32)
    return _log_softmax(logits)


# revision 21
# speedup vs baseline: 1.8151x; 1.1278x over previous
"""ChildSum TreeLSTM (N=8192 nodes, 4-ary static heap tree, H=256, D=300) on 8 trn2 NeuronCores.

Strategy
--------
The tree is static: node i's children are 4i+1..4i+4 (clipped at N). The reverse
scan (children before parents) is equivalent to processing the tree level by
level, bottom-up; nodes within a level are independent, so each level is a
batched LSTM cell (matmuls + elementwise).

Sharding: the 256 level-4 subtrees are partitioned across the 8 cores (balanced
by the number of *internal* level-6 descendants, which determines level-7 leaf
count). Each core processes its forest fully locally — children of a sorted node
range are contiguous in the next level's sorted array, so the recurrence needs
no gathers and no cross-core communication. Cores output their 32 level-4 root
(h, c) states; the tiny top of the tree (levels 3..0, 85 nodes) plus the final
log_softmax run on the host in numpy.

On-device layout: everything is transposed — feature dim on SBUF partitions
(256 features = 2 halves of 128), nodes along the free axis. The child-h sums
and per-child forget gates then become strided slicing along the free axis.
Key perf choices (round 2):
  * bf16 weights + h-state: avoids the f32r 4x small-matmul penalty on the
    recurrence matmuls, halves weight DMA, and unlocks DVE 2x/4x modes.
  * inputs repacked on host to [128, k, cols] so each (queue, priority) chunk
    is ONE large DMA; 9 input DMAs total spread over 4 queues, ordered so the
    first range's operands land first.
  * x-side projections in three uniform 352-column ranges; the forget gate is
    only computed for the 256 internal-node columns (leaves don't use it).
  * leaf-column gate activations are applied directly while draining PSUM
    (act-drain) — raw gx never round-trips through SBUF for leaf columns.
  * biases (bx + bh, zeros in practice) are folded into an extra ones-row of
    the x-side matmul, so pad columns (zero x) self-compute to h = c = 0.
"""

import numpy as np
import ml_dtypes

BF16 = ml_dtypes.bfloat16
FP8 = ml_dtypes.float8_e4m3fn

N = 8192
H = 256
D = 300
K = 4
OUT = 4
NCORES = 8
L7P = 384           # padded level-7 columns per core (4 * IPMAX)
IPMAX = 96          # max internal level-6 nodes per core
KUSE = 301          # contraction rows actually used (300 emb + 1 ones)
XCOLS = L7P + 512 + 128   # 1024 per-core node columns: [L7 | L6 | L5]
XS = 32.0           # fp8 quantization scale for embeddings
WS = 8.0            # fp8 quantization scale for Wx
DESCALE = 1.0 / (XS * WS)

GATE_MAP = [0, 2, 3, 1]  # our gate order [i, o, u, f] -> reference gate indices

F32 = np.float32


def _build_plan():
    """Assign the 256 level-4 subtrees to 8 cores; build per-core column maps."""
    # w(u) = number of internal (has-children) level-6 descendants of L4 node u.
    # Full-weight subtrees (w=16) are u in [85, 127); u=127 has w=11; rest 0.
    full = list(range(85, 127))                               # 42 subtrees
    lights = list(range(128, 341))                            # 213 subtrees
    heavy_counts = [6, 6, 5, 5, 5, 5, 5, 5]                   # sums to 42
    light_counts = [26, 26, 26, 27, 27, 27, 27, 27]           # sums to 213
    cores = []
    hpos = 0
    lpos = 0
    for c in range(NCORES):
        hs = full[hpos:hpos + heavy_counts[c]]
        hpos += heavy_counts[c]
        if c == 2:
            hs = hs + [127]                                   # w sums: 96,96,91,80*5
        ls = lights[lpos:lpos + light_counts[c]]
        lpos += light_counts[c]
        cores.append(sorted(hs + ls))
    all_l4 = sorted(u for cs in cores for u in cs)
    assert all_l4 == list(range(85, 341)), "L4 assignment must partition [85, 341)"

    plan = []
    for c in range(NCORES):
        l4 = cores[c]
        assert len(l4) == 32
        l5 = [4 * u + 1 + k for u in l4 for k in range(K)]
        l6 = [4 * v + 1 + k for v in l5 for k in range(K)]
        wc = sum(1 for x in l6 if x < 2048)
        assert wc <= IPMAX
        l7 = []
        for x in l6[:wc]:
            for k in range(K):
                ch = 4 * x + 1 + k
                l7.append(ch if ch < N else -1)
        l7 += [-1] * (L7P - len(l7))
        cols = np.array(l7 + l6 + l5, dtype=np.int64)
        assert cols.shape == (XCOLS,)
        plan.append((cols, wc, np.array(l5, dtype=np.int64)))
    return plan


_PLAN = _build_plan()

# chunk schedule: (xoff, ncols, ip, child_level, child_col_off, out_level,
# out_off, goff).  goff = column offset of this chunk's internal nodes inside
# the 224-wide internal-gx tiles (GXI / GX3): [L6int 0:96 | L5 96:224].
# Level 4 and above run on the host.
_CHUNKS = [
    (0,    256, 0,   None, 0,   7, 0,   None),   # L7 leaves, part A
    (256,  128, 0,   None, 0,   7, 256, None),   # L7 leaves, part B
    (640,  256, 0,   None, 0,   6, 256, None),   # L6 leaf-only half
    (384,  256, 96,  7,    0,   6, 0,   0),      # L6 internal half
    (896,  128, 128, 6,    0,   5, 0,   96),     # L5
]
_STATE_COLS = {7: L7P, 6: 512, 5: 128}

# phase-1 column ranges (each <= 512 PSUM f32 columns, uniform for ramping)
_RANGES = [(0, 352), (352, 704), (704, 1024)]
# per range: internal windows (copy-drained to GXI / matmul'd for the f
# gate), with dst offset in the 224-wide internal tiles.
_INT_WIN = [[], [(384, 480, 0)], [(896, 1024, 96)]]


def _static_tree():
    idx = np.arange(N)[:, None] * K + 1 + np.arange(K)[None, :]
    mask = (idx < N).astype(F32)
    idx = np.where(idx < N, idx, 0).astype(np.int32)
    return idx, mask


_STATIC_IDX, _STATIC_MASK = _static_tree()


def _pack_weights(Wx, bx, Wh, bh):
    """Pack to partition-major HBM layouts.

    wx3  [128, 3, 1024] fp8e4m3 (x WS): wx3[p, k, 256*g + j] = Wx[rg][j, 128k+p]
                               row 301 (k=2, p=45) holds (bx+bh) * WS.
    wh2  [128, 2, 768]  bf16 : i/o/u recurrence weights, transposed.
    whf2 [128, 2, 256]  bf16 : forget recurrence weights, transposed.
    """
    wx = np.zeros((384, 4 * H), dtype=F32)
    for g, rg in enumerate(GATE_MAP):
        wx[:D, H * g:H * (g + 1)] = np.asarray(Wx[rg], dtype=F32).T
        wx[D, H * g:H * (g + 1)] = np.asarray(bx[rg], dtype=F32) + np.asarray(bh[rg], dtype=F32)
    wx3 = np.ascontiguousarray(
        (wx * WS).reshape(3, 128, 4 * H).transpose(1, 0, 2)).astype(FP8)
    wh = np.zeros((H, 3 * H), dtype=F32)
    for g, rg in enumerate([0, 2, 3]):  # i, o, u
        wh[:, H * g:H * (g + 1)] = np.asarray(Wh[rg], dtype=F32).T
    wh2 = np.ascontiguousarray(wh.reshape(2, 128, 3 * H).transpose(1, 0, 2)).astype(BF16)
    whf = np.asarray(Wh[1], dtype=F32).T
    whf2 = np.ascontiguousarray(whf.reshape(2, 128, H).transpose(1, 0, 2)).astype(BF16)
    return wx3, wh2, whf2


def _pack_xt(xs, emb_table):
    """Per-core transposed embeddings, partition-major: [128, 3, XCOLS] fp8."""
    X = np.asarray(emb_table, dtype=F32)[np.asarray(xs)]
    xts = []
    for cols, _, _ in _PLAN:
        xt = np.zeros((384, XCOLS), dtype=F32)
        real = cols >= 0
        xt[:D, real] = X[cols[real]].T
        xt[D, real] = 1.0
        xt3 = np.ascontiguousarray(
            (xt * XS).reshape(3, 128, XCOLS).transpose(1, 0, 2)).astype(FP8)
        xts.append(xt3)
    return xts


def _sigmoid(x):
    return (1.0 / (1.0 + np.exp(-x))).astype(F32)


def _host_top(Hbuf, Cbuf, xs, emb_table, Wx, bx, Wh, bh):
    """Compute tree levels 4..0 (nodes 0..340) on the host, numpy fp32."""
    Wx = np.asarray(Wx, dtype=F32)
    bx = np.asarray(bx, dtype=F32)
    Wh = np.asarray(Wh, dtype=F32)
    bh = np.asarray(bh, dtype=F32)
    emb = np.asarray(emb_table, dtype=F32)
    xs = np.asarray(xs)
    for lo, hi in [(85, 341), (21, 85), (5, 21), (1, 5), (0, 1)]:
        ids = np.arange(lo, hi)
        Xl = emb[xs[ids]]                                   # [n, D]
        gx = np.einsum('ghd,nd->ngh', Wx, Xl).astype(F32) + bx
        cidx = ids[:, None] * K + 1 + np.arange(K)[None, :]  # all valid (< 341)
        Hc = Hbuf[cidx]
        Cc = Cbuf[cidx]
        hs = Hc.sum(1)
        ig = _sigmoid(gx[:, 0] + hs @ Wh[0].T + bh[0])
        og = _sigmoid(gx[:, 2] + hs @ Wh[2].T + bh[2])
        ug = np.tanh(gx[:, 3] + hs @ Wh[3].T + bh[3]).astype(F32)
        f = _sigmoid(gx[:, 1][:, None, :] + Hc @ Wh[1].T + bh[1])
        cc = ig * ug + (f * Cc).sum(1)
        hh = og * np.tanh(cc).astype(F32)
        Hbuf[ids] = hh
        Cbuf[ids] = cc
    return Hbuf[0]


def _log_softmax(x):
    m = np.max(x)
    e = np.exp(x - m)
    return (x - m - np.log(e.sum())).astype(F32)


# ----------------------------------------------------------------------------
# Bass device program
# ----------------------------------------------------------------------------

_COMPILED = None


def _build_device_program():
    import contextlib

    import concourse.bacc as bacc
    import concourse.tile as tile
    import concourse.mybir as mybir

    f32 = mybir.dt.float32
    bf16 = mybir.dt.bfloat16
    fp8 = mybir.dt.float8e4
    Sig = mybir.ActivationFunctionType.Sigmoid
    Tanh = mybir.ActivationFunctionType.Tanh
    AxX = mybir.AxisListType.X
    Add = mybir.AluOpType.add
    DRow = mybir.MatmulPerfMode.DoubleRow

    nc = bacc.Bacc("TRN2", target_bir_lowering=False, debug=False,
                   num_devices=NCORES)

    xt_d = nc.dram_tensor("xt", [128, 3, XCOLS], fp8, kind="ExternalInput")
    wx_d = nc.dram_tensor("wx", [128, 3, 4 * H], fp8, kind="ExternalInput")
    wh_d = nc.dram_tensor("wh", [128, 2, 3 * H], bf16, kind="ExternalInput")
    whf_d = nc.dram_tensor("whf", [128, 2, H], bf16, kind="ExternalInput")
    out_h_d = nc.dram_tensor("out_h", [128, 2, 128], bf16, kind="ExternalOutput")
    out_c_d = nc.dram_tensor("out_c", [128, 2, 128], bf16, kind="ExternalOutput")

    GFUNC = [Sig, Sig, Tanh]   # activation per gate i, o, u

    with tile.TileContext(nc) as tc:
        with contextlib.ExitStack() as ctx:
            inp = ctx.enter_context(tc.tile_pool(name="inp", bufs=1))
            st = ctx.enter_context(tc.tile_pool(name="state", bufs=1))
            wk = ctx.enter_context(tc.tile_pool(name="work", bufs=2))
            ps = ctx.enter_context(
                tc.tile_pool(name="psum", bufs=1, space="PSUM"))

            # --- input SBUF tiles (single tile per tensor; sliced DMAs)
            xt_s = inp.tile([128, 3, XCOLS], fp8, tag="xt", name="xt")
            wx_s = inp.tile([128, 3, 4 * H], fp8, tag="wx", name="wx")
            wh_s = inp.tile([128, 2, 3 * H], bf16, tag="wh", name="wh")
            whf_s = inp.tile([128, 2, H], bf16, tag="whf", name="whf")

            # priority DMAs (only sync / scalar / gpsimd queues can issue
            # DMAs): range 0 split by k across the three queues, with the
            # first weight quarter leading on scalar.
            a0, b0 = _RANGES[0]
            nc.scalar.dma_start(out=wx_s[:, :, 0:256], in_=wx_d[:, :, 0:256])
            nc.sync.dma_start(out=xt_s[:, 0, a0:b0], in_=xt_d[:, 0, a0:b0])
            nc.scalar.dma_start(out=xt_s[:, 1, a0:b0], in_=xt_d[:, 1, a0:b0])
            nc.gpsimd.dma_start(out=xt_s[:, 2, a0:b0], in_=xt_d[:, 2, a0:b0])
            # remaining ranges / quarters, less urgent
            a1, b1 = _RANGES[1]
            a2, b2 = _RANGES[2]
            nc.sync.dma_start(out=xt_s[:, :, a1:b1], in_=xt_d[:, :, a1:b1])
            nc.scalar.dma_start(out=xt_s[:, :, a2:b2], in_=xt_d[:, :, a2:b2])
            for q in range(1, 4):
                nc.gpsimd.dma_start(out=wx_s[:, :, 256 * q:256 * (q + 1)],
                                    in_=wx_d[:, :, 256 * q:256 * (q + 1)])
            nc.sync.dma_start(out=wh_s[:], in_=wh_d[:])
            nc.sync.dma_start(out=whf_s[:], in_=whf_d[:])

            # --- persistent state + gate tiles
            SH = {lv: st.tile([128, 2, n], bf16, tag=f"h{lv}", name=f"sh{lv}")
                  for lv, n in _STATE_COLS.items()}
            SC = {lv: st.tile([128, 2, n], bf16, tag=f"c{lv}", name=f"sc{lv}")
                  for lv, n in _STATE_COLS.items()}
            # activated gates for ALL columns (leaf cols filled by act-drain
            # in phase 1; internal cols filled during phase 2)
            G = [st.tile([128, 2, XCOLS], bf16, tag=f"g{g}", name=f"g{g}")
                 for g in range(3)]
            # raw gx for internal columns only (i/o/u for the psum add; f for
            # the per-child forget bias)
            GXI = [st.tile([128, 2, 224], f32, tag=f"gxi{g}", name=f"gxi{g}")
                   for g in range(3)]
            GX3 = st.tile([128, 2, 224], f32, tag="gx3", name="gx3")

            with nc.allow_low_precision("bf16 gates/h-state within 2e-2 tol"):
                # --- phase 1: x-side projections, range-major sweep.
                # leaf windows act-drained straight into G; internal windows
                # copy-drained into GXI; f-gate computed only on internal
                # windows into a single psum tile, drained to GX3.
                gxf = ps.tile([128, 2, 224], f32, tag="gxf", name="gxf")
                for ri, (a, b) in enumerate(_RANGES):
                    for m in range(6):
                        g, phi = divmod(m, 2)
                        col = 128 * m
                        pt = ps.tile([128, 512], f32, tag="gx", bufs=3,
                                     name=f"pgx{ri}_{m}")
                        # fp8 DoubleRow: k-subtiles 0+1 in one half-rate pass,
                        # the 45-row remainder (k=2, incl. bias row) single.
                        nc.tensor.matmul(
                            pt[:, 0:b - a],
                            wx_s[:, 0:2, col:col + 128],
                            xt_s[:, 0:2, a:b],
                            start=True, stop=False, perf_mode=DRow)
                        nc.tensor.matmul(
                            pt[:, 0:b - a],
                            wx_s[:, 2, col:col + 128],
                            xt_s[:, 2, a:b],
                            start=False, stop=True)
                        # act-drain the whole range into G with the fp8
                        # descale folded into the activation's input scale
                        # (internal cols are overwritten by phase 2); raw gx
                        # for internal cols is copy-drained separately.
                        nc.scalar.activation(
                            G[g][:, phi, a:b], pt[:, 0:b - a], GFUNC[g],
                            scale=DESCALE)
                        for (wa, wb, go) in _INT_WIN[ri]:
                            nc.vector.tensor_scalar_mul(
                                GXI[g][:, phi, go:go + wb - wa],
                                pt[:, wa - a:wb - a], DESCALE)
                    for (wa, wb, go) in _INT_WIN[ri]:
                        for phi in range(2):
                            col = 768 + 128 * phi
                            nc.tensor.matmul(
                                gxf[:, phi, go:go + wb - wa],
                                wx_s[:, 0:2, col:col + 128],
                                xt_s[:, 0:2, wa:wb],
                                start=True, stop=False, perf_mode=DRow)
                            nc.tensor.matmul(
                                gxf[:, phi, go:go + wb - wa],
                                wx_s[:, 2, col:col + 128],
                                xt_s[:, 2, wa:wb],
                                start=False, stop=True)
                nc.vector.tensor_scalar_mul(GX3[:], gxf[:], DESCALE)

                # --- phase 2: levels bottom-up in chunks
                for (xoff, cn, ip, child, coff, outlv, ooff, goff) in _CHUNKS:
                    if ip > 0:
                        # hs = sum of the 4 child h columns per node
                        hs = wk.tile([128, 2, ip], bf16, tag="hs", name="hs")
                        cv = SH[child][:, :, coff:coff + 4 * ip].rearrange(
                            "p t (n k) -> p t n k", k=K)
                        nc.vector.tensor_reduce(hs[:], cv, axis=AxX, op=Add)

                        # i/o/u gates: preload gx into PSUM (off the critical
                        # chain), then accumulate wh @ hs on top of it.
                        Ps = []
                        for g in range(3):
                            P = ps.tile([128, 2, 128], f32, tag="pa", bufs=2,
                                        name=f"pa{outlv}_{g}")
                            nc.vector.tensor_copy(
                                P[:, :, 0:ip], GXI[g][:, :, goff:goff + ip])
                            Ps.append(P)
                        # forget psum: preload the per-child gf broadcast
                        Pfc = ps.tile([128, 2, 512], f32, tag="pf", name="pfc")
                        pv = Pfc[:, :, 0:4 * ip].rearrange(
                            "p t (n k) -> p t n k", k=K)
                        gfb = GX3[:, :, goff:goff + ip][:, :, :, None]
                        nc.vector.tensor_copy(
                            pv, gfb.broadcast_to([128, 2, ip, K]))

                        for g in range(3):
                            P = Ps[g]
                            for phi in range(2):
                                for k in range(2):
                                    nc.tensor.matmul(
                                        P[:, phi, 0:ip],
                                        wh_s[:, k, 256 * g + 128 * phi:
                                             256 * g + 128 * phi + 128],
                                        hs[:, k, 0:ip],
                                        start=False, stop=(k == 1),
                                        skip_group_check=True)
                            nc.scalar.activation(
                                G[g][:, :, xoff:xoff + ip], P[:, :, 0:ip],
                                GFUNC[g])

                        # f = sigmoid(gf + Whf @ h_child) per child;
                        # csum = sum_k f_k * c_child_k
                        FS = wk.tile([128, 2, 512], bf16, tag="fs", name="fs")
                        for phi in range(2):
                            for k in range(2):
                                nc.tensor.matmul(
                                    Pfc[:, phi, 0:4 * ip],
                                    whf_s[:, k, 128 * phi:128 * phi + 128],
                                    SH[child][:, k, coff:coff + 4 * ip],
                                    start=False, stop=(k == 1),
                                    skip_group_check=True)
                            nc.scalar.activation(
                                FS[:, phi, 0:4 * ip], Pfc[:, phi, 0:4 * ip],
                                Sig)
                        nc.vector.tensor_mul(
                            FS[:, :, 0:4 * ip], FS[:, :, 0:4 * ip],
                            SC[child][:, :, coff:coff + 4 * ip])
                        sv = FS[:, :, 0:4 * ip].rearrange(
                            "p t (n k) -> p t n k", k=K)
                        csum = wk.tile([128, 2, ip], bf16, tag="csum",
                                       name="csum")
                        nc.vector.tensor_reduce(csum[:], sv, axis=AxX, op=Add)

                    # c = ig*ug (+ csum on internal cols); h = og*tanh(c)
                    Cdst = SC[outlv][:, :, ooff:ooff + cn]
                    nc.vector.tensor_mul(
                        Cdst, G[0][:, :, xoff:xoff + cn],
                        G[2][:, :, xoff:xoff + cn])
                    if ip > 0:
                        nc.vector.tensor_add(
                            SC[outlv][:, :, ooff:ooff + ip],
                            SC[outlv][:, :, ooff:ooff + ip],
                            csum[:])
                    TC = wk.tile([128, 2, 256], bf16, tag="tc", name="tc")
                    nc.scalar.activation(TC[:, :, 0:cn], Cdst, Tanh)
                    nc.vector.tensor_mul(
                        SH[outlv][:, :, ooff:ooff + cn],
                        G[1][:, :, xoff:xoff + cn], TC[:, :, 0:cn])

            nc.sync.dma_start(out=out_h_d[:], in_=SH[5][:])
            nc.sync.dma_start(out=out_c_d[:], in_=SC[5][:])

    nc.compile()
    return nc


def _get_compiled():
    global _COMPILED
    if _COMPILED is None:
        _COMPILED = _build_device_program()
    return _COMPILED


def _numpy_fallback(xs, child_idx, child_mask, emb_table, Wx, bx, Wh, bh,
                    Wout, bout):
    """Exact sequential scan in numpy; only used if the tree is not the
    expected static 4-ary heap."""
    X = np.asarray(emb_table, dtype=F32)[np.asarray(xs)]
    Wx = np.asarray(Wx, dtype=F32)
    Wh = np.asarray(Wh, dtype=F32)
    bx = np.asarray(bx, dtype=F32)
    bh = np.asarray(bh, dtype=F32)
    gx = np.einsum('ghd,nd->ngh', Wx, X).astype(F32) + bx
    Hb = np.zeros((N, H), dtype=F32)
    Cb = np.zeros((N, H), dtype=F32)
    ci = np.asarray(child_idx)
    cm = np.asarray(child_mask, dtype=F32)
    for i in range(N - 1, -1, -1):
        idx = ci[i]
        m = cm[i][:, None]
        Hc = Hb[idx] * m
        Cc = Cb[idx] * m
        hs = Hc.sum(0)
        g = gx[i]
        ig = _sigmoid(g[0] + Wh[0] @ hs + bh[0])
        og = _sigmoid(g[2] + Wh[2] @ hs + bh[2])
        ug = np.tanh(g[3] + Wh[3] @ hs + bh[3]).astype(F32)
        f = _sigmoid(g[1] + Hc @ Wh[1].T + bh[1])
        c = ig * ug + (f * Cc).sum(0)
        Hb[i] = og * np.tanh(c).astype(F32)
        Cb[i] = c
    logits = np.asarray(Wout, dtype=F32) @ Hb[0] + np.asarray(bout, dtype=F32)
    return _log_softmax(logits)


def kernel(xs, child_idx, child_mask, emb_table, Wx, bx, Wh, bh, Wout, bout):
    xs = np.asarray(xs)
    if not (np.array_equal(np.asarray(child_idx), _STATIC_IDX)
            and np.array_equal(np.asarray(child_mask, dtype=F32), _STATIC_MASK)):
        return _numpy_fallback(xs, child_idx, child_mask, emb_table, Wx, bx,
                               Wh, bh, Wout, bout)

    from concourse.bass_utils import run_bass_kernel_spmd

    wx3, wh2, whf2 = _pack_weights(Wx, bx, Wh, bh)
    xts = _pack_xt(xs, emb_table)
    in_maps = [
        {"xt": xts[c], "wx": wx3, "wh": wh2, "whf": whf2}
        for c in range(NCORES)
    ]
    nc = _get_compiled()
    res = run_bass_kernel_spmd(nc, in_maps, core_ids=list(range(NCORES)))

    Hbuf = np.zeros((1365, H), dtype=F32)
    Cbuf = np.zeros((1365, H), dtype=F32)
    for c in range(NCORES):
        _, _, l5 = _PLAN[c]
        oh = np.asarray(res.results[c]["out_h"], dtype=F32)   # [128, 2, 128]
        oc = np.asarray(res.results[c]["out_c"], dtype=F32)
        Hbuf[l5] = np.concatenate([oh[:, 0, :], oh[:, 1, :]], axis=0).T
        Cbuf[l5] = np.concatenate([oc[:, 0, :], oc[:, 1, :]], axis=0).T

    h0 = _host_top(Hbuf, Cbuf, xs, emb_table, Wx, bx, Wh, bh)
    logits = np.asarray(Wout, dtype=F32) @ h0 + np.asarray(bout, dtype=F32)
    return _log_softmax(logits)
